# revision 18
# baseline (speedup 1.0000x reference)
"""MoE router kernel for Trainium2 (Bass/Tile), SPMD over 8 NeuronCores.

Computes, for x:(B,T,D) f32, W:(E,D) f32, x_mask:(B,T) i32 {0,1}:
  m       = x_mask[..., None]
  logits  = (x*m) @ W.T * m            # (B,T,E)
  probs   = softmax(logits, -1)
  ew, ei  = top2(probs);  ew /= ew.sum(-1, keepdims=True);  ew *= m
  ei      = where(m, ei, -1)
  probs   = probs * m
returns (ew, ei, logits, probs).

Sharding: data-parallel over B*T tokens, 4096 tokens per core, W replicated.
Within a core, token tau = p*32 + s (p in [0,128) partitions, s in [0,32)),
so every DMA moves large contiguous runs per partition.
"""

import sys

sys.path.insert(0, "/opt/trn_rl_repo")

from contextlib import ExitStack

import numpy as np

import bass_rust as _br
import concourse.bass as bass
import concourse.mybir as mybir
import concourse.tile as tile
from concourse import bacc
from concourse.bass_utils import run_bass_kernel_spmd
from concourse.masks import make_identity

N_CORES = 8
B, T, D, E = 4, 8192, 1024, 64
P = 128              # SBUF partitions
NTOK = B * T // N_CORES   # 4096 tokens per core
S = NTOK // P        # 32 tokens per partition
SG = 4               # s-columns per pipeline group
G = S // SG          # 8 groups
DC = D // 128        # 8 contraction chunks
TOKG = P * SG        # 512 tokens per group

f32 = mybir.dt.float32
f32r = mybir.dt.float32r
bf16 = mybir.dt.bfloat16
i32 = mybir.dt.int32
u32 = mybir.dt.uint32
ACT_COPY = mybir.ActivationFunctionType.Copy
ACT_EXP = mybir.ActivationFunctionType.Exp


def build_nc(mm_f32r=True, tr_f32r=False):
    nc = bacc.Bacc("TRN2", target_bir_lowering=False, debug=False)

    x_d = nc.dram_tensor("x", [NTOK, D], f32, kind="ExternalInput").ap()
    w_d = nc.dram_tensor("w", [E, D], f32, kind="ExternalInput").ap()
    m_d = nc.dram_tensor("mask", [NTOK], i32, kind="ExternalInput").ap()
    ew_d = nc.dram_tensor("ew", [NTOK, 2], f32, kind="ExternalOutput").ap()
    ei_d = nc.dram_tensor("ei", [NTOK, 2], i32, kind="ExternalOutput").ap()
    lg_d = nc.dram_tensor("logits", [NTOK, E], f32, kind="ExternalOutput").ap()
    pr_d = nc.dram_tensor("probs", [NTOK, E], f32, kind="ExternalOutput").ap()

    x_v = x_d.rearrange("(p s) d -> p s d", p=P)
    m_v = m_d.rearrange("(p s) -> p s", p=P)
    lg_v = lg_d.rearrange("(p s) e -> p s e", p=P)
    pr_v = pr_d.rearrange("(p s) e -> p s e", p=P)
    ew_v = ew_d.rearrange("(p s) k -> p s k", p=P)
    ei_v = ei_d.rearrange("(p s) k -> p s k", p=P)

    # Walrus's fused-LW matmult struct (fp32/fp32r/transpose-mode) fits a
    # single sync wait. PE "observes" a semaphore tick only when some PE
    # instruction explicitly waits on it, so before each matmult that would
    # need two waits we insert a dummy standalone bf16 LDWEIGHTS reading the
    # dependency region: it absorbs one wait, leaving the matmult with one.
    def prewait(nc, ap, before=None):
        ld = nc.tensor.ldweights(ap.bitcast(bf16))
        return ld

    def order(after, before_ld):
        _br.add_dep_helper(after.ins, before_ld.ins, sync=False,
                           reason="pe wait-slot split")

    with tile.TileContext(nc) as tc, ExitStack() as ctx:
        const = ctx.enter_context(tc.tile_pool(name="const", bufs=1))
        persist = ctx.enter_context(tc.tile_pool(name="persist", bufs=1))
        xpool = ctx.enter_context(tc.tile_pool(name="xp", bufs=2))
        xtpool = ctx.enter_context(tc.tile_pool(name="xtp", bufs=2))
        ltpool = ctx.enter_context(tc.tile_pool(name="ltp", bufs=2))
        tailp = ctx.enter_context(tc.tile_pool(name="tailp", bufs=1))
        ps_xt = ctx.enter_context(tc.tile_pool(name="ps_xt", bufs=2, space="PSUM"))
        ps_lt = ctx.enter_context(tc.tile_pool(name="ps_lt", bufs=2, space="PSUM"))
        ps_lg = ctx.enter_context(tc.tile_pool(name="ps_lg", bufs=2, space="PSUM"))

        mmdt = f32r if mm_f32r else f32

        ident = const.tile([P, P], f32)
        make_identity(nc, ident)
        # Warm-up transpose reading only ident: PE observes the Pool semaphore
        # here, so later transposes never need a second wait slot for it
        # (walrus's fused-LW matmult struct fits a single sync wait).
        pwm = ps_lg.tile([E, P], f32, tag="pslg")
        nc.tensor.transpose(pwm[:], ident[:, 0:E], ident[:])

        # ---- W -> WT chunks: wt[:, c, e] = W[e, c*128 + p] ----
        # wt/xt are written as float32r: the writing engine rounds to the
        # fp32r-representable set, which lets the PE run the gating matmul
        # at full rate (1 cyc/row) instead of fp32's 4 cyc/row.
        w_sb = const.tile([E, D], f32)
        nc.sync.dma_start(w_sb[:], w_d)
        wt = const.tile([P, DC, E], mmdt)
        for c in range(DC):
            ld = None
            if c >= 2:
                ld = prewait(nc, wt[:, c - 2, 0:4])
            pw = ps_lg.tile([P, E], f32, tag="pslg")
            t = nc.tensor.transpose(pw[:], w_sb[:, c * 128:(c + 1) * 128],
                                    ident[0:E, 0:E])
            if ld is not None:
                order(t, ld)
            nc.scalar.activation(wt[:, c, :], pw[:], ACT_COPY)

        # ---- mask -> f32 m_sb (P,S); mm1 = m-1 ----
        m_i = const.tile([P, S], i32)
        nc.sync.dma_start(m_i[:], m_v)
        m_sb = const.tile([P, S], f32)
        nc.vector.tensor_copy(m_sb[:], m_i[:])
        mm1 = const.tile([P, S], f32)
        nc.vector.tensor_scalar_add(mm1[:], m_sb[:], -1.0)

        e_sb = persist.tile([P, S, E], f32)    # exp(masked logits)
        sums = persist.tile([P, S], f32)       # softmax denominators
        lg_sb = persist.tile([P, S, E], f32)   # masked logits (output)
        mx_all = persist.tile([P, S, 8], f32)  # top-8 values per token
        ix_all = persist.tile([P, S, 8], u32)  # top-8 indices per token

        for g in range(G):
            s0 = g * SG
            xg = xpool.tile([P, SG, D], f32, tag="xg")
            nc.gpsimd.dma_start(xg[:], x_v[:, s0:s0 + SG, :])
            # xt free layout per s: (c, tok_p); chunk c rhs = [:, :, c*128:+128]
            xt = xtpool.tile([P, SG, D], mmdt, tag="xt")
            # All xt/wt/lt producers stay on ACT: every PE wait in the steady
            # state is against the single ACT semaphore (or PE's own).
            ld_dma = prewait(nc, xg[:, 0, 0:4])
            for sl in range(SG):
                ld = ld_dma if sl == 0 else None
                if sl >= 2:
                    ld = prewait(nc, xt[:, sl - 2, 0:4])
                pxt = ps_xt.tile([P, D], f32, tag="psxt")
                for c in range(DC):
                    t = nc.tensor.transpose(
                        pxt[:, c * 128:(c + 1) * 128],
                        xg[:, sl, c * 128:(c + 1) * 128],
                        ident[:])
                    if c == 0 and ld is not None:
                        order(t, ld)
                nc.scalar.activation(xt[:, sl, :], pxt[:], ACT_COPY)

            ld2 = prewait(nc, xt[:, SG - 2, 0:4])
            ld3 = prewait(nc, xt[:, SG - 1, 0:4])
            plt = ps_lt.tile([E, TOKG], f32, tag="pslt")  # logits.T (64, 512)
            for c in range(DC):
                m = nc.tensor.matmul(
                    plt[:],
                    wt[:, c, :],
                    xt[:, :, c * 128:(c + 1) * 128],
                    start=(c == 0), stop=(c == DC - 1))
                if c == 0:
                    order(m, ld2)
                    order(m, ld3)
            lt_sb = ltpool.tile([E, TOKG], f32, tag="lt")
            nc.scalar.activation(lt_sb[:], plt[:], ACT_COPY)

            ld_lt = prewait(nc, lt_sb[0:E, 0:4])
            for sl in range(SG):
                s_abs = s0 + sl
                ld = ld_lt if sl == 0 else None
                if sl >= 2:
                    ld = prewait(nc, e_sb[:, s0 + sl - 2, 0:4])
                plg = ps_lg.tile([P, E], f32, tag="pslg")  # logits (128, 64)
                t = nc.tensor.transpose(plg[:], lt_sb[:, sl * 128:(sl + 1) * 128],
                                        ident[0:E, 0:E])
                if ld is not None:
                    order(t, ld)
                # Both plg readers stay on ACT so the PSUM slot release is a
                # single semaphore for the next back-transpose's one wait slot.
                mcol = m_sb[:, s_abs:s_abs + 1]
                nc.scalar.activation(lg_sb[:, s_abs, :], plg[:], ACT_COPY,
                                     scale=mcol)
                nc.scalar.activation(e_sb[:, s_abs, :], plg[:], ACT_EXP,
                                     scale=mcol,
                                     accum_out=sums[:, s_abs:s_abs + 1])
                nc.vector.max(mx_all[:, s_abs, :], e_sb[:, s_abs, :])
                nc.vector.max_index(ix_all[:, s_abs, :], mx_all[:, s_abs, :],
                                    e_sb[:, s_abs, :])

        # ---- batched tail ----
        inv = tailp.tile([P, S], f32)
        nc.vector.reciprocal(inv[:], sums[:])
        rm = tailp.tile([P, S], f32)
        nc.vector.tensor_mul(rm[:], inv[:], m_sb[:])
        probs_sb = tailp.tile([P, S, E], f32)
        nc.vector.tensor_mul(probs_sb[:], e_sb[:],
                             rm[:].unsqueeze(2).broadcast_to([P, S, E]))

        s12 = tailp.tile([P, S], f32)
        nc.vector.tensor_add(s12[:], mx_all[:, :, 0], mx_all[:, :, 1])
        r12 = tailp.tile([P, S], f32)
        nc.vector.reciprocal(r12[:], s12[:])
        rw = tailp.tile([P, S], f32)
        nc.vector.tensor_mul(rw[:], r12[:], m_sb[:])
        ew_sb = tailp.tile([P, S, 2], f32)
        nc.vector.tensor_mul(ew_sb[:, :, 0], mx_all[:, :, 0], rw[:])
        nc.vector.tensor_mul(ew_sb[:, :, 1], mx_all[:, :, 1], rw[:])

        if_sb = tailp.tile([P, S, 2], f32)
        nc.vector.tensor_copy(if_sb[:], ix_all[:, :, 0:2])
        nc.vector.tensor_mul(if_sb[:], if_sb[:],
                             m_sb[:].unsqueeze(2).broadcast_to([P, S, 2]))
        nc.vector.tensor_add(if_sb[:], if_sb[:],
                             mm1[:].unsqueeze(2).broadcast_to([P, S, 2]))
        ei_sb = tailp.tile([P, S, 2], i32)
        nc.vector.tensor_copy(ei_sb[:], if_sb[:])

        nc.sync.dma_start(lg_v, lg_sb[:])
        nc.sync.dma_start(pr_v, probs_sb[:])
        nc.sync.dma_start(ew_v, ew_sb[:])
        nc.sync.dma_start(ei_v, ei_sb[:])

    nc.compile()
    return nc


_NC_CACHE = {}


def get_nc(mm_f32r=True, tr_f32r=True):
    key = (mm_f32r, tr_f32r)
    if key not in _NC_CACHE:
        _NC_CACHE[key] = build_nc(mm_f32r=mm_f32r, tr_f32r=tr_f32r)
    return _NC_CACHE[key]


def make_in_maps(x, W, x_mask):
    x = np.ascontiguousarray(np.asarray(x, dtype=np.float32).reshape(B * T, D))
    W = np.ascontiguousarray(np.asarray(W, dtype=np.float32))
    m = np.ascontiguousarray(np.asarray(x_mask, dtype=np.int32).reshape(B * T))
    in_maps = []
    for c in range(N_CORES):
        in_maps.append({
            "x": np.ascontiguousarray(x[c * NTOK:(c + 1) * NTOK]),
            "w": W,
            "mask": np.ascontiguousarray(m[c * NTOK:(c + 1) * NTOK]),
        })
    return in_maps


def assemble(results):
    ew = np.concatenate([r["ew"] for r in results], axis=0)
    ei = np.concatenate([r["ei"] for r in results], axis=0)
    lg = np.concatenate([r["logits"] for r in results], axis=0)
    pr = np.concatenate([r["probs"] for r in results], axis=0)
    return (
        ew.reshape(B, T, 2).astype(np.float32),
        ei.reshape(B, T, 2).astype(np.int32),
        lg.reshape(B, T, E).astype(np.float32),
        pr.reshape(B, T, E).astype(np.float32),
    )


def kernel(x, W, x_mask):
    nc = get_nc()
    in_maps = make_in_maps(x, W, x_mask)
    res = run_bass_kernel_spmd(nc, in_maps, list(range(N_CORES))).results
    return assemble(res)


# revision 20
# speedup vs baseline: 1.2093x; 1.2093x over previous
"""MoE router kernel for Trainium2 (Bass/Tile), SPMD over 8 NeuronCores.

Computes, for x:(B,T,D) f32, W:(E,D) f32, x_mask:(B,T) i32 {0,1}:
  m       = x_mask[..., None]
  logits  = (x*m) @ W.T * m            # (B,T,E)
  probs   = softmax(logits, -1)
  ew, ei  = top2(probs);  ew /= ew.sum(-1, keepdims=True);  ew *= m
  ei      = where(m, ei, -1)
  probs   = probs * m
returns (ew, ei, logits, probs).

Sharding: data-parallel over B*T tokens, 4096 tokens per core, W replicated.
Layout strategy: the host pre-transposes each core's x shard to d-major
(1024, 4096) so the contraction dim lands on SBUF partitions with perfectly
contiguous DMA — no on-chip transposes of x are needed, and the TensorE only
runs the gating matmul in exact fp32 (2-way column tiling: two half-K
accumulations in the two PSUM partition halves, summed on the DVE).

Within a core, token tau = n*128 + p (n tile 0..31, p partition); outputs are
written tile-major (p, n, e) and unscrambled on the host.
"""

import sys

sys.path.insert(0, "/opt/trn_rl_repo")

from contextlib import ExitStack

import numpy as np

import concourse.bass as bass
import concourse.mybir as mybir
import concourse.tile as tile
from concourse import bacc
from concourse.bass_utils import run_bass_kernel_spmd
from concourse.masks import make_identity

N_CORES = 8
B, T, D, E = 4, 8192, 1024, 64
P = 128                   # SBUF partitions
NTOK = B * T // N_CORES   # 4096 tokens per core
S = NTOK // P             # 32 token-tiles (tokens per partition)
SG = 4                    # token-tiles per pipeline group
G = S // SG               # 8 groups
DC = D // 128             # 8 contraction chunks
TOKG = P * SG             # 512 tokens per group

f32 = mybir.dt.float32
i32 = mybir.dt.int32
u32 = mybir.dt.uint32
ACT_COPY = mybir.ActivationFunctionType.Copy
ACT_EXP = mybir.ActivationFunctionType.Exp


def build_nc():
    nc = bacc.Bacc("TRN2", target_bir_lowering=False, debug=False)

    xt_d = nc.dram_tensor("xt", [D, NTOK], f32, kind="ExternalInput").ap()
    w_d = nc.dram_tensor("w", [E, D], f32, kind="ExternalInput").ap()
    m_d = nc.dram_tensor("mask", [P, S], i32, kind="ExternalInput").ap()
    ew_d = nc.dram_tensor("ew", [P, S, 2], f32, kind="ExternalOutput").ap()
    ei_d = nc.dram_tensor("ei", [P, S, 2], i32, kind="ExternalOutput").ap()
    lg_d = nc.dram_tensor("logits", [P, S, E], f32, kind="ExternalOutput").ap()
    pr_d = nc.dram_tensor("probs", [P, S, E], f32, kind="ExternalOutput").ap()

    xt_v = xt_d.rearrange("(c p) t -> p c t", p=P)   # (128, 8, 4096)

    with tile.TileContext(nc) as tc, ExitStack() as ctx:
        const = ctx.enter_context(tc.tile_pool(name="const", bufs=1))
        persist = ctx.enter_context(tc.tile_pool(name="persist", bufs=1))
        xpool = ctx.enter_context(tc.tile_pool(name="xp", bufs=2))
        ltpool = ctx.enter_context(tc.tile_pool(name="ltp", bufs=2))
        tailp = ctx.enter_context(tc.tile_pool(name="tailp", bufs=1))
        ps_lt = ctx.enter_context(tc.tile_pool(name="ps_lt", bufs=2, space="PSUM"))
        ps_lg = ctx.enter_context(tc.tile_pool(name="ps_lg", bufs=2, space="PSUM"))

        ident = const.tile([P, P], f32)
        make_identity(nc, ident)
        # Warm-up transpose reading only ident: PE observes the Pool semaphore
        # early so real matmults keep their single fused-LW wait slot free.
        pwm = ps_lg.tile([E, P], f32, tag="pslg")
        nc.tensor.transpose(pwm[:], ident[:, 0:E], ident[:])

        # ---- W -> WT chunks: wt[:, c, e] = W[e, c*128 + p] ----
        w_sb = const.tile([E, D], f32)
        nc.sync.dma_start(w_sb[:], w_d)
        wt = const.tile([P, DC, E], f32)
        for c in range(DC):
            pw = ps_lg.tile([P, E], f32, tag="pslg")
            nc.tensor.transpose(pw[:], w_sb[:, c * 128:(c + 1) * 128],
                                ident[0:E, 0:E])
            nc.scalar.activation(wt[:, c, :], pw[:], ACT_COPY)

        # ---- mask (pre-laid-out host-side as (p, n)) -> f32; mm1 = m-1 ----
        m_i = const.tile([P, S], i32)
        nc.sync.dma_start(m_i[:], m_d)
        m_sb = const.tile([P, S], f32)
        nc.vector.tensor_copy(m_sb[:], m_i[:])
        mm1 = const.tile([P, S], f32)
        nc.vector.tensor_scalar_add(mm1[:], m_sb[:], -1.0)

        e_sb = persist.tile([P, S, E], f32)    # exp(masked logits)
        sums = persist.tile([P, S], f32)       # softmax denominators
        lg_sb = persist.tile([P, S, E], f32)   # masked logits (output)
        mx_all = persist.tile([P, S, 8], f32)  # top-8 values per token
        ix_all = persist.tile([P, S, 8], u32)  # top-8 indices per token

        for g in range(G):
            t0 = g * TOKG
            xtg = xpool.tile([P, DC, TOKG], f32, tag="xtg")
            nc.sync.dma_start(xtg[:], xt_v[:, :, t0:t0 + TOKG])

            # Exact fp32 gating matmul, 2-way column tiling: chunks 0-3
            # accumulate logits.T into PSUM partitions 0-63, chunks 4-7 into
            # 64-127; the halves run in different array column groups and are
            # summed on the DVE.
            plt = ps_lt.tile([P, TOKG], f32, tag="pslt")
            for ci in range(DC // 2):
                for h in range(2):
                    c = h * (DC // 2) + ci
                    nc.tensor.matmul(
                        plt[h * E:(h + 1) * E, :],
                        wt[:, c, :],
                        xtg[:, c, :],
                        start=(ci == 0), stop=(ci == DC // 2 - 1),
                        tile_position=(0, h * E),
                        skip_group_check=True)
            lt_a = ltpool.tile([E, TOKG], f32, tag="lta")
            nc.scalar.activation(lt_a[:], plt[0:E, :], ACT_COPY)
            lt_sb = ltpool.tile([E, TOKG], f32, tag="lt")
            nc.vector.tensor_add(lt_sb[:], lt_a[:], plt[E:P, :])

            for sl in range(SG):
                s_abs = g * SG + sl
                plg = ps_lg.tile([P, E], f32, tag="pslg")  # logits (128, 64)
                nc.tensor.transpose(plg[:], lt_sb[:, sl * 128:(sl + 1) * 128],
                                    ident[0:E, 0:E])
                mcol = m_sb[:, s_abs:s_abs + 1]
                nc.scalar.activation(lg_sb[:, s_abs, :], plg[:], ACT_COPY,
                                     scale=mcol)
                nc.scalar.activation(e_sb[:, s_abs, :], plg[:], ACT_EXP,
                                     scale=mcol,
                                     accum_out=sums[:, s_abs:s_abs + 1])
                nc.vector.max(mx_all[:, s_abs, :], e_sb[:, s_abs, :])
                nc.vector.max_index(ix_all[:, s_abs, :], mx_all[:, s_abs, :],
                                    e_sb[:, s_abs, :])

        # ---- batched tail ----
        inv = tailp.tile([P, S], f32)
        nc.vector.reciprocal(inv[:], sums[:])
        rm = tailp.tile([P, S], f32)
        nc.vector.tensor_mul(rm[:], inv[:], m_sb[:])
        probs_sb = tailp.tile([P, S, E], f32)
        nc.vector.tensor_mul(probs_sb[:], e_sb[:],
                             rm[:].unsqueeze(2).broadcast_to([P, S, E]))

        s12 = tailp.tile([P, S], f32)
        nc.vector.tensor_add(s12[:], mx_all[:, :, 0], mx_all[:, :, 1])
        r12 = tailp.tile([P, S], f32)
        nc.vector.reciprocal(r12[:], s12[:])
        rw = tailp.tile([P, S], f32)
        nc.vector.tensor_mul(rw[:], r12[:], m_sb[:])
        ew_sb = tailp.tile([P, S, 2], f32)
        nc.vector.tensor_mul(ew_sb[:, :, 0], mx_all[:, :, 0], rw[:])
        nc.vector.tensor_mul(ew_sb[:, :, 1], mx_all[:, :, 1], rw[:])

        if_sb = tailp.tile([P, S, 2], f32)
        nc.vector.tensor_copy(if_sb[:], ix_all[:, :, 0:2])
        nc.vector.tensor_mul(if_sb[:], if_sb[:],
                             m_sb[:].unsqueeze(2).broadcast_to([P, S, 2]))
        nc.vector.tensor_add(if_sb[:], if_sb[:],
                             mm1[:].unsqueeze(2).broadcast_to([P, S, 2]))
        ei_sb = tailp.tile([P, S, 2], i32)
        nc.vector.tensor_copy(ei_sb[:], if_sb[:])

        nc.sync.dma_start(lg_d, lg_sb[:])
        nc.sync.dma_start(pr_d, probs_sb[:])
        nc.sync.dma_start(ew_d, ew_sb[:])
        nc.sync.dma_start(ei_d, ei_sb[:])

    nc.compile()
    return nc


_NC_CACHE = {}


def get_nc():
    if "nc" not in _NC_CACHE:
        _NC_CACHE["nc"] = build_nc()
    return _NC_CACHE["nc"]


def make_in_maps(x, W, x_mask):
    x = np.asarray(x, dtype=np.float32).reshape(B * T, D)
    W = np.ascontiguousarray(np.asarray(W, dtype=np.float32))
    m = np.asarray(x_mask, dtype=np.int32).reshape(B * T)
    in_maps = []
    for c in range(N_CORES):
        xs = x[c * NTOK:(c + 1) * NTOK]                    # (4096, 1024)
        ms = m[c * NTOK:(c + 1) * NTOK]                    # (4096,)
        in_maps.append({
            "xt": np.ascontiguousarray(xs.T),              # (1024, 4096)
            "w": W,
            # token tau = n*128 + p  ->  mask tile [p, n]
            "mask": np.ascontiguousarray(ms.reshape(S, P).T),
        })
    return in_maps


def _unscramble(a):
    # kernel writes (p, n, k); token tau = n*128 + p
    return a.transpose(1, 0, 2).reshape(NTOK, a.shape[2])


def assemble(results):
    ew = np.concatenate([_unscramble(r["ew"]) for r in results], axis=0)
    ei = np.concatenate([_unscramble(r["ei"]) for r in results], axis=0)
    lg = np.concatenate([_unscramble(r["logits"]) for r in results], axis=0)
    pr = np.concatenate([_unscramble(r["probs"]) for r in results], axis=0)
    return (
        np.ascontiguousarray(ew.reshape(B, T, 2), dtype=np.float32),
        np.ascontiguousarray(ei.reshape(B, T, 2), dtype=np.int32),
        np.ascontiguousarray(lg.reshape(B, T, E), dtype=np.float32),
        np.ascontiguousarray(pr.reshape(B, T, E), dtype=np.float32),
    )


def kernel(x, W, x_mask):
    nc = get_nc()
    in_maps = make_in_maps(x, W, x_mask)
    res = run_bass_kernel_spmd(nc, in_maps, list(range(N_CORES))).results
    return assemble(res)
